# revision 10
# baseline (speedup 1.0000x reference)
"""Mixtral MoE block (B=4,S=2048,H=1024,F=3584,E=8,K=2,cap=640) on 8 TRN2 cores.

Strategy: expert-parallel. Each core c owns expert c (gets w1/w2/w3 slices as
its inputs) and computes, on-device:
  1. router logits/softmax for ALL tokens (replicated),
  2. fused-order (n=2s+k) capacity-slot assignment via triangular-matmul cumsum,
  3. slot inversion (token-id/prob per capacity slot) via dma_scatter_add,
  4. token gather (dma_gather), PE-transpose, SwiGLU FFN in fp32r,
  5. weighted scatter-add of expert outputs into a per-core partial output.
Host sums the 8 partial outputs (each token receives exactly K=2 expert
contributions across cores; all other rows are zero).
"""
import sys
import numpy as np

sys.path.insert(0, '/opt/trn_rl_repo')

B, S, H, F, E, K = 4, 2048, 1024, 3584, 8, 2
T = B * S                  # 8192 tokens
CAP = 640                  # int(S*K/E*1.25)
HC = H // 128              # 8 h-chunks
FC = F // 128              # 28 f-chunks
NTT = T // 128             # 64 token tiles
BLK = 32                   # fused blocks per batch (4096/128)
NF = 2 * S                 # fused positions per batch = 4096
DUMP = E * CAP // 2        # 2560: dump row in slotD
SLOT_ROWS = DUMP + 128     # 2688
BIG = 1.0e30

_cached = None


def _build():
    from concourse import bacc
    import concourse.mybir as mybir
    import concourse.tile as tile
    from concourse.masks import make_identity, make_upper_triangular

    f32 = mybir.dt.float32
    f32r = mybir.dt.float32r
    bf16 = mybir.dt.bfloat16
    i16 = mybir.dt.int16
    AL = mybir.AluOpType
    ACTF = mybir.ActivationFunctionType

    nc = bacc.Bacc("TRN2", num_devices=8, num_swdge_queues=2)

    hsD = nc.dram_tensor("hs", [T, H], f32, kind="ExternalInput")
    gwD = nc.dram_tensor("gate_w", [E, H], f32, kind="ExternalInput")
    w1D = nc.dram_tensor("w1e", [H, F], f32, kind="ExternalInput")
    w2D = nc.dram_tensor("w2e", [F, H], f32, kind="ExternalInput")
    w3D = nc.dram_tensor("w3e", [H, F], f32, kind="ExternalInput")
    eselD = nc.dram_tensor("esel", [128, BLK * E], f32, kind="ExternalInput")
    kparD = nc.dram_tensor("kpar", [128, 2], f32, kind="ExternalInput")  # col0: even(k=0) mask, col1: odd
    tqD = nc.dram_tensor("tqiota", [128, BLK], f32, kind="ExternalInput")  # blk*64 + p//2

    outD = nc.dram_tensor("out", [T + 1, H], f32, kind="ExternalOutput")

    from contextlib import ExitStack
    with ExitStack() as ctx:
        tc = ctx.enter_context(tile.TileContext(nc))
        dpool = ctx.enter_context(tc.tile_pool(name="dram", bufs=1, space="DRAM"))
        cpool = ctx.enter_context(tc.tile_pool(name="const", bufs=1))
        xpool = ctx.enter_context(tc.tile_pool(name="x", bufs=2))
        spool = ctx.enter_context(tc.tile_pool(name="small", bufs=2))
        rpool = ctx.enter_context(tc.tile_pool(name="route", bufs=1))
        w13s_pool = ctx.enter_context(tc.tile_pool(name="w13s", bufs=2))
        w13r_pool = ctx.enter_context(tc.tile_pool(name="w13r", bufs=2))
        w2s_pool = ctx.enter_context(tc.tile_pool(name="w2s", bufs=2))
        w2r_pool = ctx.enter_context(tc.tile_pool(name="w2r", bufs=2))
        xd_pool = ctx.enter_context(tc.tile_pool(name="xd", bufs=2))
        xdt_pool = ctx.enter_context(tc.tile_pool(name="xdt", bufs=1))
        g_pool = ctx.enter_context(tc.tile_pool(name="g", bufs=1))
        on_pool = ctx.enter_context(tc.tile_pool(name="on", bufs=1))
        ps_tp = ctx.enter_context(tc.tile_pool(name="ps_tp", bufs=2, space="PSUM"))
        ps_mm = ctx.enter_context(tc.tile_pool(name="ps_mm", bufs=3, space="PSUM"))
        ps_sm = ctx.enter_context(tc.tile_pool(name="ps_sm", bufs=3, space="PSUM"))
        if True:

            # ---------------- DRAM scratch ----------------
            expTD = dpool.tile([E, T], f32)       # exp(logits).T
            denD = dpool.tile([T], f32)           # softmax denominators
            slotD = dpool.tile([SLOT_ROWS, 64], f32)  # [b*CAP+c] -> [tok-8192, prob, ...]
            scxD = dpool.tile([B, NF], f32)       # scatter dest idx per fused pos

            # ---------------- constants ----------------
            ident = cpool.tile([128, 128], f32)
            make_identity(nc, ident[:])
            triu = cpool.tile([128, 128], f32)
            make_upper_triangular(nc, triu[:])
            ones8 = cpool.tile([8, 1], f32)
            nc.vector.memset(ones8[:], 1.0)
            ones1x = cpool.tile([1, 128], f32)
            nc.vector.memset(ones1x[:], 1.0)
            ones128 = cpool.tile([128, 1], f32)
            nc.vector.memset(ones128[:], 1.0)
            eself = cpool.tile([128, BLK * E], f32)
            nc.sync.dma_start(out=eself[:], in_=eselD[:])
            kpar = cpool.tile([128, 2], f32)
            nc.sync.dma_start(out=kpar[:], in_=kparD[:])
            tqio = cpool.tile([128, BLK], f32)
            nc.sync.dma_start(out=tqio[:], in_=tqD[:])

            # zero slotD
            zsb = cpool.tile([128, 64], f32)
            nc.vector.memset(zsb[:], 0.0)
            for r in range(0, SLOT_ROWS, 128):
                nc.sync.dma_start(out=slotD[r:r + 128, :], in_=zsb[:])

            # gate_w.T in sbuf: gwT[:, hc*8:(hc+1)*8] = gate_w[:, hc*128:(hc+1)*128].T
            gnat = cpool.tile([E, H], f32)
            nc.sync.dma_start(out=gnat[:], in_=gwD[:])
            gwT = cpool.tile([128, HC * E], f32)
            for hc in range(HC):
                tp = ps_tp.tile([128, 128], f32, tag="tp")
                nc.tensor.transpose(out=tp[:128, :8], in_=gnat[:, hc * 128:(hc + 1) * 128],
                                    identity=ident[0:8, 0:8])
                nc.vector.tensor_copy(out=gwT[:, hc * 8:(hc + 1) * 8], in_=tp[:128, :8])

            # ---------------- phase 1: logits -> exp, denom ----------------
            for t in range(NTT):
                xt = xpool.tile([128, H], f32, tag="xt")
                nc.sync.dma_start(out=xt[:], in_=hsD[t * 128:(t + 1) * 128, :])
                lg = ps_sm.tile([128, 256], f32, tag="sm")
                for hc in range(HC):
                    tp = ps_tp.tile([128, 128], f32, tag="tp")
                    nc.tensor.transpose(out=tp[:], in_=xt[:, hc * 128:(hc + 1) * 128],
                                        identity=ident[:])
                    xtT = xpool.tile([128, 128], f32, tag="xtT")
                    nc.vector.tensor_copy(out=xtT[:], in_=tp[:])
                    nc.tensor.matmul(out=lg[:8, :128], lhsT=gwT[:, hc * 8:(hc + 1) * 8],
                                     rhs=xtT[:], start=(hc == 0), stop=(hc == HC - 1))
                ex = xpool.tile([8, 128], f32, tag="ex")
                nc.scalar.activation(out=ex[:], in_=lg[:8, :128], func=ACTF.Exp)
                dn = ps_sm.tile([128, 256], f32, tag="sm")
                nc.tensor.matmul(out=dn[:1, :128], lhsT=ones8[:], rhs=ex[:], start=True, stop=True)
                dns = xpool.tile([1, 128], f32, tag="dns")
                nc.vector.tensor_copy(out=dns[:], in_=dn[:1, :128])
                nc.sync.dma_start(out=expTD[:, t * 128:(t + 1) * 128], in_=ex[:])
                nc.sync.dma_start(out=denD[t * 128:(t + 1) * 128], in_=dns[0:1, :])

            # ---------------- phase 2: routing per batch ----------------
            PAY = rpool.tile([128, 4 * BLK, 2], f32, tag="pay")
            exp_v = expTD[:].rearrange("e (b blk j) -> b j blk e", b=B, blk=BLK, j=64)
            den_v = denD[:].rearrange("(b blk j) -> b j blk", b=B, blk=BLK, j=64)
            for b in range(B):
                E2 = rpool.tile([128, BLK, E], f32, tag="E2")
                D2 = rpool.tile([128, BLK], f32, tag="D2")
                for par in (0, 1):
                    for e in range(E):
                        nc.sync.dma_start(out=E2[par:128:2, :, e], in_=exp_v[b, :, :, e])
                    nc.sync.dma_start(out=D2[par:128:2, :], in_=den_v[b])
                R2 = rpool.tile([128, BLK], f32, tag="R2")
                nc.vector.reciprocal(out=R2[:], in_=D2[:])
                P2 = rpool.tile([128, BLK, E], f32, tag="P2")
                nc.vector.tensor_tensor(out=P2[:], in0=E2[:], in1=R2[:, :, None].to_broadcast([128, BLK, E]), op=AL.mult)

                # top-1 / top-2 one-hots from E2 (monotone in logits)
                m1 = rpool.tile([128, BLK], f32, tag="m1")
                nc.vector.tensor_copy(out=m1[:], in_=E2[:, :, 0])
                for e in range(1, E):
                    nc.vector.tensor_tensor(out=m1[:], in0=m1[:], in1=E2[:, :, e], op=AL.max)
                oh1 = rpool.tile([128, BLK, E], f32, tag="oh1")
                nc.vector.tensor_tensor(out=oh1[:], in0=E2[:], in1=m1[:, :, None].to_broadcast([128, BLK, E]), op=AL.is_equal)
                E2b = rpool.tile([128, BLK, E], f32, tag="E2b")
                nc.vector.scalar_tensor_tensor(out=E2b[:], in0=oh1[:], scalar=-BIG, in1=E2[:],
                                               op0=AL.mult, op1=AL.add)
                m2 = rpool.tile([128, BLK], f32, tag="m2")
                nc.vector.tensor_copy(out=m2[:], in_=E2b[:, :, 0])
                for e in range(1, E):
                    nc.vector.tensor_tensor(out=m2[:], in0=m2[:], in1=E2b[:, :, e], op=AL.max)
                oh2 = rpool.tile([128, BLK, E], f32, tag="oh2")
                nc.vector.tensor_tensor(out=oh2[:], in0=E2b[:], in1=m2[:, :, None].to_broadcast([128, BLK, E]), op=AL.is_equal)

                FM = rpool.tile([128, BLK * E], f32, tag="FM")
                oh1f = oh1[:].rearrange("p blk e -> p (blk e)")
                oh2f = oh2[:].rearrange("p blk e -> p (blk e)")
                t1 = rpool.tile([128, BLK * E], f32, tag="t1")
                nc.vector.tensor_scalar(out=FM[:], in0=oh1f, scalar1=kpar[:, 0:1], scalar2=None, op0=AL.mult)
                nc.vector.tensor_scalar(out=t1[:], in0=oh2f, scalar1=kpar[:, 1:2], scalar2=None, op0=AL.mult)
                nc.vector.tensor_add(out=FM[:], in0=FM[:], in1=t1[:])

                # cumulative count over fused order
                cnt_ps = ps_sm.tile([128, 256], f32, tag="sm")
                nc.tensor.matmul(out=cnt_ps[:], lhsT=triu[:], rhs=FM[:], start=True, stop=False)
                tot_ps = ps_sm.tile([128, 256], f32, tag="sm")
                nc.tensor.matmul(out=tot_ps[:1, :], lhsT=ones128[:],
                                 rhs=FM[:], start=True, stop=True)
                # exclusive prefix over blocks (on partition 0)
                pfA = rpool.tile([1, 256], f32, tag="pfA")
                pfB = rpool.tile([1, 256], f32, tag="pfB")
                tot = rpool.tile([1, 256], f32, tag="tot")
                nc.vector.tensor_copy(out=tot[:], in_=tot_ps[:1, :])
                nc.vector.tensor_copy(out=pfA[:], in_=tot[:])
                cur, nxt = pfA, pfB
                for sh in (1, 2, 4, 8, 16):
                    w = sh * E
                    nc.vector.tensor_add(out=nxt[0:1, w:256], in0=cur[0:1, w:256], in1=cur[0:1, 0:256 - w])
                    nc.vector.tensor_copy(out=nxt[0:1, 0:w], in_=cur[0:1, 0:w])
                    cur, nxt = nxt, cur
                offs = rpool.tile([1, 256], f32, tag="offs")
                nc.vector.tensor_sub(out=offs[:], in0=cur[:], in1=tot[:])
                nc.tensor.matmul(out=cnt_ps[:], lhsT=ones1x[:1, :], rhs=offs[:], start=False, stop=True)

                cnt = rpool.tile([128, BLK * E], f32, tag="cnt")
                nc.vector.tensor_copy(out=cnt[:], in_=cnt_ps[:])
                keep = rpool.tile([128, BLK * E], f32, tag="keep")
                nc.vector.tensor_scalar(out=keep[:], in0=cnt[:], scalar1=CAP + 0.5, scalar2=None, op0=AL.is_le)
                trunc = rpool.tile([128, BLK * E], f32, tag="trunc")
                nc.vector.tensor_mul(out=trunc[:], in0=FM[:], in1=keep[:])
                mske = rpool.tile([128, BLK * E], f32, tag="mske")
                nc.vector.tensor_mul(out=mske[:], in0=trunc[:], in1=eself[:])
                slot1 = rpool.tile([128, BLK * E], f32, tag="slot1")
                nc.vector.tensor_mul(out=slot1[:], in0=cnt[:], in1=mske[:])
                pm = rpool.tile([128, BLK * E], f32, tag="pm")
                nc.vector.tensor_mul(out=pm[:], in0=P2[:].rearrange("p blk e -> p (blk e)"), in1=mske[:])

                def ereduce(dst, srcf, tag):
                    sv = srcf.rearrange("p (blk e) -> p blk e", e=E)
                    nc.vector.tensor_copy(out=dst[:], in_=sv[:, :, 0])
                    for e in range(1, E):
                        nc.vector.tensor_add(out=dst[:], in0=dst[:], in1=sv[:, :, e])

                myslot = rpool.tile([128, BLK], f32, tag="myslot")
                ereduce(myslot, slot1[:], "ms")
                mine = rpool.tile([128, BLK], f32, tag="mine")
                ereduce(mine, mske[:], "mn")
                myprob = rpool.tile([128, BLK], f32, tag="mp")
                ereduce(myprob, pm[:], "mr")

                # payload: [tok-8192, prob]; dest: b*CAP+slot-1 (mine) else DUMP
                nc.vector.tensor_scalar(out=PAY[:, b * BLK:(b + 1) * BLK, 0],
                                        in0=tqio[:], scalar1=float(b * S - T), scalar2=None, op0=AL.add)
                nc.vector.tensor_copy(out=PAY[:, b * BLK:(b + 1) * BLK, 1], in_=myprob[:])
                sd = rpool.tile([128, BLK], f32, tag="sd")
                nc.vector.tensor_scalar(out=sd[:], in0=myslot[:], scalar1=float(b * CAP - 1 - DUMP), scalar2=None, op0=AL.add)
                nc.vector.tensor_mul(out=sd[:], in0=sd[:], in1=mine[:])
                nc.vector.tensor_scalar(out=sd[:], in0=sd[:], scalar1=float(DUMP), scalar2=None, op0=AL.add)
                nc.sync.dma_start(out=scxD[b, :].rearrange("(blk p) -> p blk", p=128), in_=sd[:])

            # inversion scatters (per batch)
            for b in range(B):
                scw = spool.tile([128, NF // 16], f32, tag="scw")
                src = scxD[b, :].rearrange("(j c) -> c j", c=16)
                for r in range(8):
                    nc.sync.dma_start(out=scw[r * 16:(r + 1) * 16, :], in_=src)
                scw16 = spool.tile([128, NF // 16], i16, tag="scw16")
                nc.vector.tensor_copy(out=scw16[:], in_=scw[:])
                nc.gpsimd.dma_scatter_add(
                    out_ap=slotD[:, 0:2], in_ap=PAY[:, b * BLK:(b + 1) * BLK, :],
                    idxs_ap=scw16[:], num_idxs=NF, num_idxs_reg=NF,
                    elem_size=2, elem_step=64, queue_num=1)

            # ---------------- phase 3: FFN per batch-supertile ----------------
            for b in range(B):
                # slot metadata
                gidf = spool.tile([128, CAP // 16], f32, tag="gidf")
                for r in range(8):
                    nc.sync.dma_start(
                        out=gidf[r * 16:(r + 1) * 16, :],
                        in_=slotD[b * CAP:(b + 1) * CAP, 0].rearrange("(j c) -> c j", c=16))
                sidf = spool.tile([128, CAP // 16], f32, tag="sidf")
                nc.vector.tensor_scalar(out=sidf[:], in0=gidf[:], scalar1=float(T), scalar2=None, op0=AL.add)
                sid16 = spool.tile([128, CAP // 16], i16, tag="sid16")
                nc.vector.tensor_copy(out=sid16[:], in_=sidf[:])
                gidc = spool.tile([128, CAP // 16], f32, tag="gidc")
                nc.vector.tensor_scalar(out=gidc[:], in0=sidf[:], scalar1=float(T - 1), scalar2=None, op0=AL.min)
                gid16 = spool.tile([128, CAP // 16], i16, tag="gid16")
                nc.vector.tensor_copy(out=gid16[:], in_=gidc[:])
                prb = spool.tile([128, CAP // 128], f32, tag="prb")
                nc.sync.dma_start(out=prb[:], in_=slotD[b * CAP:(b + 1) * CAP, 1].rearrange("(j p) -> p j", p=128))

                # gather + transpose
                XdT = xdt_pool.tile([128, HC, CAP], f32r, tag="xdt")
                for j in range(CAP // 128):
                    xdn = xd_pool.tile([128, 1, H], f32, tag="xdn")
                    nc.gpsimd.dma_gather(
                        out_ap=xdn[:], in_ap=hsD[:], idxs_ap=gid16[:, j * 8:(j + 1) * 8],
                        num_idxs=128, num_idxs_reg=128, elem_size=H, queue_num=0)
                    for hc in range(HC):
                        tp = ps_tp.tile([128, 128], f32, tag="tp")
                        nc.tensor.transpose(out=tp[:], in_=xdn[:, 0, hc * 128:(hc + 1) * 128], identity=ident[:])
                        nc.vector.tensor_copy(out=XdT[:, hc, j * 128:(j + 1) * 128], in_=tp[:])

                # pass 1: g = silu(x@w1) * (x@w3), fp32r
                g = g_pool.tile([128, FC, CAP], bf16, tag="g")
                for fc in range(FC):
                    w1s = w13s_pool.tile([128, HC, 128], f32, tag="w1s")
                    nc.sync.dma_start(out=w1s[:], in_=w1D[:, fc * 128:(fc + 1) * 128].rearrange("(hc p) f -> p hc f", p=128))
                    w1r = w13r_pool.tile([128, HC, 128], f32r, tag="w1r")
                    nc.scalar.copy(out=w1r[:], in_=w1s[:])
                    w3s = w13s_pool.tile([128, HC, 128], f32, tag="w3s")
                    nc.sync.dma_start(out=w3s[:], in_=w3D[:, fc * 128:(fc + 1) * 128].rearrange("(hc p) f -> p hc f", p=128))
                    w3r = w13r_pool.tile([128, HC, 128], f32r, tag="w3r")
                    nc.scalar.copy(out=w3r[:], in_=w3s[:])
                    for c0 in range(0, CAP, 320):
                        ps1 = ps_mm.tile([128, 320], f32, tag="mm")
                        ps3 = ps_mm.tile([128, 320], f32, tag="mm")
                        for hc in range(HC):
                            nc.tensor.matmul(out=ps1[:], lhsT=w1r[:, hc, :], rhs=XdT[:, hc, c0:c0 + 320],
                                             start=(hc == 0), stop=(hc == HC - 1))
                        for hc in range(HC):
                            nc.tensor.matmul(out=ps3[:], lhsT=w3r[:, hc, :], rhs=XdT[:, hc, c0:c0 + 320],
                                             start=(hc == 0), stop=(hc == HC - 1))
                        sil = xpool.tile([128, 320], f32, tag="sil")
                        nc.scalar.activation(out=sil[:], in_=ps1[:], func=ACTF.Silu)
                        nc.vector.tensor_tensor(out=g[:, fc, c0:c0 + 320], in0=sil[:], in1=ps3[:], op=AL.mult)

                # pass 2: out = g @ w2, transpose back, scale by prob, scatter
                on_t = on_pool.tile([128, CAP // 128, H], f32, tag="on")
                for hc in range(HC):
                    oT = xpool.tile([128, CAP], f32, tag="oT")
                    for half in range(2):
                        fch = FC // 2  # 14
                        w2s = w2s_pool.tile([128, FC // 2, 128], f32, tag="w2s")
                        nc.sync.dma_start(
                            out=w2s[:],
                            in_=w2D[half * fch * 128:(half + 1) * fch * 128, hc * 128:(hc + 1) * 128]
                                .rearrange("(fc p) h -> p fc h", p=128))
                        w2r = w2r_pool.tile([128, FC // 2, 128], bf16, tag="w2r")
                        nc.scalar.copy(out=w2r[:], in_=w2s[:])
                        for c0 in range(0, CAP, 320):
                            pso = ps_mm.tile([128, 320], f32, tag="mm")
                            for fq in range(fch):
                                fc = half * fch + fq
                                nc.tensor.matmul(out=pso[:], lhsT=w2r[:, fq, :], rhs=g[:, fc, c0:c0 + 320],
                                                 start=(fq == 0), stop=(fq == fch - 1))
                            if half == 0:
                                nc.vector.tensor_copy(out=oT[:, c0:c0 + 320], in_=pso[:])
                            else:
                                nc.vector.tensor_add(out=oT[:, c0:c0 + 320], in0=oT[:, c0:c0 + 320], in1=pso[:])
                    for j in range(CAP // 128):
                        tpo = ps_tp.tile([128, 128], f32, tag="tp")
                        otr = xpool.tile([128, 128], f32, tag="otr")
                        nc.vector.tensor_copy(out=otr[:], in_=oT[:, j * 128:(j + 1) * 128])
                        nc.tensor.transpose(out=tpo[:], in_=otr[:], identity=ident[:])
                        nc.vector.tensor_scalar(out=on_t[:, j, hc * 128:(hc + 1) * 128],
                                                in0=tpo[:], scalar1=prb[:, j:j + 1], scalar2=None, op0=AL.mult)
                nc.gpsimd.dma_scatter_add(
                    out_ap=outD[:], in_ap=on_t[:], idxs_ap=sid16[:],
                    num_idxs=CAP, num_idxs_reg=CAP, elem_size=H, queue_num=1)

    nc.finalize()
    return nc


def _host_consts():
    esel = np.zeros((8, 128, BLK * E), np.float32)
    for c in range(8):
        esel[c, :, c::E] = 1.0
    kpar = np.zeros((128, 2), np.float32)
    kpar[0::2, 0] = 1.0
    kpar[1::2, 1] = 1.0
    tq = np.zeros((128, BLK), np.float32)
    p = np.arange(128)
    for blk in range(BLK):
        tq[:, blk] = blk * 64 + p // 2
    return esel, kpar, tq


def kernel(**inputs):
    global _cached
    from concourse.bass_utils import run_bass_kernel_spmd
    if _cached is None:
        _cached = _build()
    nc = _cached

    hs = np.ascontiguousarray(inputs["hidden_states"].reshape(T, H).astype(np.float32, copy=False))
    gw = np.ascontiguousarray(inputs["gate_w"].astype(np.float32, copy=False))
    w1 = inputs["w1"]
    w2 = inputs["w2"]
    w3 = inputs["w3"]
    esel, kpar, tq = _host_consts()

    in_maps = []
    for c in range(8):
        in_maps.append({
            "hs": hs,
            "gate_w": gw,
            "w1e": np.ascontiguousarray(w1[c].astype(np.float32, copy=False)),
            "w2e": np.ascontiguousarray(w2[c].astype(np.float32, copy=False)),
            "w3e": np.ascontiguousarray(w3[c].astype(np.float32, copy=False)),
            "esel": esel[c],
            "kpar": kpar,
            "tqiota": tq,
        })
    import os
    trace = bool(int(os.environ.get("KERNEL_TRACE", "0")))
    kw = {}
    if trace:
        import types
        if "antenv.axon_hooks" not in sys.modules:
            import antenv
            m = types.ModuleType("antenv.axon_hooks")
            holder = {}
            m.set_axon_ntff_profile_hook = lambda h: holder.__setitem__("h", h)
            m.get_axon_ntff_profile_hook = lambda: holder.get("h")
            sys.modules["antenv.axon_hooks"] = m
            antenv.axon_hooks = m
            from trn_agent_boot.trn_boot import _ntff_profile_via_ctypes
            h = _ntff_profile_via_ctypes('/opt/axon/libaxon_pjrt.so')
            m.set_axon_ntff_profile_hook(h)
        kw = dict(trace=True, trace_cores=[int(x) for x in os.environ.get("KERNEL_TRACE_CORES", "0").split(",")], stitch_traces=False)
    res = run_bass_kernel_spmd(nc, in_maps, core_ids=list(range(8)), **kw)
    if trace:
        print("exec_time_ns:", res.exec_time_ns, "mean:", res.mean_exec_time_ns,
              "max core:", res.max_exec_time_core_id)
        if res.per_core_scope_times:
            for scope, d in sorted(res.per_core_scope_times.items()):
                print("scope", scope, {k: f"{v/1000:.0f}us" for k, v in sorted(d.items())})
        if res.instructions_and_trace:
            print("trace:", res.instructions_and_trace[1])
        globals()["last_perf"] = res
    acc = np.zeros((T, H), np.float32)
    for c in range(8):
        acc += res.results[c]["out"][:T]
    return acc.reshape(B, S, H)


# revision 11
# speedup vs baseline: 1.0247x; 1.0247x over previous
"""Mixtral MoE block (B=4,S=2048,H=1024,F=3584,E=8,K=2,cap=640) on 8 TRN2 cores.

Strategy: expert-parallel. Each core c owns expert c (gets w1/w2/w3 slices as
its inputs) and computes, on-device:
  1. router logits/softmax for ALL tokens (replicated),
  2. fused-order (n=2s+k) capacity-slot assignment via triangular-matmul cumsum,
  3. slot inversion (token-id/prob per capacity slot) via dma_scatter_add,
  4. token gather (dma_gather), PE-transpose, SwiGLU FFN in fp32r,
  5. weighted scatter-add of expert outputs into a per-core partial output.
Host sums the 8 partial outputs (each token receives exactly K=2 expert
contributions across cores; all other rows are zero).
"""
import sys
import numpy as np

sys.path.insert(0, '/opt/trn_rl_repo')

B, S, H, F, E, K = 4, 2048, 1024, 3584, 8, 2
T = B * S                  # 8192 tokens
CAP = 640                  # int(S*K/E*1.25)
HC = H // 128              # 8 h-chunks
FC = F // 128              # 28 f-chunks
NTT = T // 128             # 64 token tiles
BLK = 32                   # fused blocks per batch (4096/128)
NF = 2 * S                 # fused positions per batch = 4096
DUMP = E * CAP // 2        # 2560: dump row in slotD
SLOT_ROWS = DUMP + 128     # 2688
BIG = 1.0e30

_cached = None


def _build():
    from concourse import bacc
    import concourse.mybir as mybir
    import concourse.tile as tile
    from concourse.masks import make_identity, make_upper_triangular

    f32 = mybir.dt.float32
    f32r = mybir.dt.float32r
    bf16 = mybir.dt.bfloat16
    i16 = mybir.dt.int16
    AL = mybir.AluOpType
    ACTF = mybir.ActivationFunctionType

    nc = bacc.Bacc("TRN2", num_devices=8, num_swdge_queues=4)

    hsD = nc.dram_tensor("hs", [T, H], f32, kind="ExternalInput")
    gwD = nc.dram_tensor("gate_w", [E, H], f32, kind="ExternalInput")
    w1D = nc.dram_tensor("w1e", [H, F], f32, kind="ExternalInput")
    w2D = nc.dram_tensor("w2e", [F, H], f32, kind="ExternalInput")
    w3D = nc.dram_tensor("w3e", [H, F], f32, kind="ExternalInput")
    eselD = nc.dram_tensor("esel", [128, BLK * E], f32, kind="ExternalInput")
    kparD = nc.dram_tensor("kpar", [128, 2], f32, kind="ExternalInput")  # col0: even(k=0) mask, col1: odd
    tqD = nc.dram_tensor("tqiota", [128, BLK], f32, kind="ExternalInput")  # blk*64 + p//2

    outD = nc.dram_tensor("out", [T + 1, H], f32, kind="ExternalOutput")

    from contextlib import ExitStack
    with ExitStack() as ctx:
        tc = ctx.enter_context(tile.TileContext(nc))
        dpool = ctx.enter_context(tc.tile_pool(name="dram", bufs=1, space="DRAM"))
        cpool = ctx.enter_context(tc.tile_pool(name="const", bufs=1))
        xpool = ctx.enter_context(tc.tile_pool(name="x", bufs=2))
        spool = ctx.enter_context(tc.tile_pool(name="small", bufs=2))
        rpool = ctx.enter_context(tc.tile_pool(name="route", bufs=1))
        w13s_pool = ctx.enter_context(tc.tile_pool(name="w13s", bufs=2))
        w13r_pool = ctx.enter_context(tc.tile_pool(name="w13r", bufs=2))
        w2s_pool = ctx.enter_context(tc.tile_pool(name="w2s", bufs=2))
        w2r_pool = ctx.enter_context(tc.tile_pool(name="w2r", bufs=2))
        xd_pool = ctx.enter_context(tc.tile_pool(name="xd", bufs=2))
        xdt_pool = ctx.enter_context(tc.tile_pool(name="xdt", bufs=1))
        g_pool = ctx.enter_context(tc.tile_pool(name="g", bufs=1))
        on_pool = ctx.enter_context(tc.tile_pool(name="on", bufs=1))
        ps_tp = ctx.enter_context(tc.tile_pool(name="ps_tp", bufs=2, space="PSUM"))
        ps_mm = ctx.enter_context(tc.tile_pool(name="ps_mm", bufs=3, space="PSUM"))
        ps_sm = ctx.enter_context(tc.tile_pool(name="ps_sm", bufs=3, space="PSUM"))
        if True:

            # ---------------- DRAM scratch ----------------
            expTD = dpool.tile([E, T], f32)       # exp(logits).T
            denD = dpool.tile([T], f32)           # softmax denominators
            slotD = dpool.tile([SLOT_ROWS, 64], f32)  # [b*CAP+c] -> [tok-8192, prob, ...]
            scxD = dpool.tile([B, NF], f32)       # scatter dest idx per fused pos

            # ---------------- constants ----------------
            ident = cpool.tile([128, 128], f32)
            make_identity(nc, ident[:])
            triu = cpool.tile([128, 128], f32)
            make_upper_triangular(nc, triu[:])
            ones8 = cpool.tile([8, 1], f32)
            nc.vector.memset(ones8[:], 1.0)
            ones1x = cpool.tile([1, 128], f32)
            nc.vector.memset(ones1x[:], 1.0)
            ones128 = cpool.tile([128, 1], f32)
            nc.vector.memset(ones128[:], 1.0)
            eself = cpool.tile([128, BLK * E], f32)
            nc.sync.dma_start(out=eself[:], in_=eselD[:])
            kpar = cpool.tile([128, 2], f32)
            nc.sync.dma_start(out=kpar[:], in_=kparD[:])
            tqio = cpool.tile([128, BLK], f32)
            nc.sync.dma_start(out=tqio[:], in_=tqD[:])

            # zero slotD
            zsb = cpool.tile([128, 64], f32)
            nc.vector.memset(zsb[:], 0.0)
            for r in range(0, SLOT_ROWS, 128):
                nc.sync.dma_start(out=slotD[r:r + 128, :], in_=zsb[:])

            # gate_w.T in sbuf: gwT[:, hc*8:(hc+1)*8] = gate_w[:, hc*128:(hc+1)*128].T
            gnat = cpool.tile([E, H], f32)
            nc.sync.dma_start(out=gnat[:], in_=gwD[:])
            gwT = cpool.tile([128, HC * E], f32)
            for hc in range(HC):
                tp = ps_tp.tile([128, 128], f32, tag="tp")
                nc.tensor.transpose(out=tp[:128, :8], in_=gnat[:, hc * 128:(hc + 1) * 128],
                                    identity=ident[0:8, 0:8])
                nc.vector.tensor_copy(out=gwT[:, hc * 8:(hc + 1) * 8], in_=tp[:128, :8])

            # ---------------- phase 1: logits -> exp, denom ----------------
            for t in range(NTT):
                xt = xpool.tile([128, H], f32, tag="xt")
                nc.sync.dma_start(out=xt[:], in_=hsD[t * 128:(t + 1) * 128, :])
                lg = ps_sm.tile([128, 256], f32, tag="sm")
                for hc in range(HC):
                    tp = ps_tp.tile([128, 128], f32, tag="tp")
                    nc.tensor.transpose(out=tp[:], in_=xt[:, hc * 128:(hc + 1) * 128],
                                        identity=ident[:])
                    xtT = xpool.tile([128, 128], f32, tag="xtT")
                    nc.vector.tensor_copy(out=xtT[:], in_=tp[:])
                    nc.tensor.matmul(out=lg[:8, :128], lhsT=gwT[:, hc * 8:(hc + 1) * 8],
                                     rhs=xtT[:], start=(hc == 0), stop=(hc == HC - 1))
                ex = xpool.tile([8, 128], f32, tag="ex")
                nc.scalar.activation(out=ex[:], in_=lg[:8, :128], func=ACTF.Exp)
                dn = ps_sm.tile([128, 256], f32, tag="sm")
                nc.tensor.matmul(out=dn[:1, :128], lhsT=ones8[:], rhs=ex[:], start=True, stop=True)
                dns = xpool.tile([1, 128], f32, tag="dns")
                nc.vector.tensor_copy(out=dns[:], in_=dn[:1, :128])
                nc.sync.dma_start(out=expTD[:, t * 128:(t + 1) * 128], in_=ex[:])
                nc.sync.dma_start(out=denD[t * 128:(t + 1) * 128], in_=dns[0:1, :])

            # ---------------- phase 2: routing per batch ----------------
            PAY = rpool.tile([128, 4 * BLK, 2], f32, tag="pay")
            exp_v = expTD[:].rearrange("e (b blk j) -> b j blk e", b=B, blk=BLK, j=64)
            den_v = denD[:].rearrange("(b blk j) -> b j blk", b=B, blk=BLK, j=64)
            for b in range(B):
                E2 = rpool.tile([128, BLK, E], f32, tag="E2")
                D2 = rpool.tile([128, BLK], f32, tag="D2")
                for par in (0, 1):
                    for e in range(E):
                        nc.sync.dma_start(out=E2[par:128:2, :, e], in_=exp_v[b, :, :, e])
                    nc.sync.dma_start(out=D2[par:128:2, :], in_=den_v[b])
                R2 = rpool.tile([128, BLK], f32, tag="R2")
                nc.vector.reciprocal(out=R2[:], in_=D2[:])
                P2 = rpool.tile([128, BLK, E], f32, tag="P2")
                nc.vector.tensor_tensor(out=P2[:], in0=E2[:], in1=R2[:, :, None].to_broadcast([128, BLK, E]), op=AL.mult)

                # top-1 / top-2 one-hots from E2 (monotone in logits)
                m1 = rpool.tile([128, BLK], f32, tag="m1")
                nc.vector.tensor_copy(out=m1[:], in_=E2[:, :, 0])
                for e in range(1, E):
                    nc.vector.tensor_tensor(out=m1[:], in0=m1[:], in1=E2[:, :, e], op=AL.max)
                oh1 = rpool.tile([128, BLK, E], f32, tag="oh1")
                nc.vector.tensor_tensor(out=oh1[:], in0=E2[:], in1=m1[:, :, None].to_broadcast([128, BLK, E]), op=AL.is_equal)
                E2b = rpool.tile([128, BLK, E], f32, tag="E2b")
                nc.vector.scalar_tensor_tensor(out=E2b[:], in0=oh1[:], scalar=-BIG, in1=E2[:],
                                               op0=AL.mult, op1=AL.add)
                m2 = rpool.tile([128, BLK], f32, tag="m2")
                nc.vector.tensor_copy(out=m2[:], in_=E2b[:, :, 0])
                for e in range(1, E):
                    nc.vector.tensor_tensor(out=m2[:], in0=m2[:], in1=E2b[:, :, e], op=AL.max)
                oh2 = rpool.tile([128, BLK, E], f32, tag="oh2")
                nc.vector.tensor_tensor(out=oh2[:], in0=E2b[:], in1=m2[:, :, None].to_broadcast([128, BLK, E]), op=AL.is_equal)

                FM = rpool.tile([128, BLK * E], f32, tag="FM")
                oh1f = oh1[:].rearrange("p blk e -> p (blk e)")
                oh2f = oh2[:].rearrange("p blk e -> p (blk e)")
                t1 = rpool.tile([128, BLK * E], f32, tag="t1")
                nc.vector.tensor_scalar(out=FM[:], in0=oh1f, scalar1=kpar[:, 0:1], scalar2=None, op0=AL.mult)
                nc.vector.tensor_scalar(out=t1[:], in0=oh2f, scalar1=kpar[:, 1:2], scalar2=None, op0=AL.mult)
                nc.vector.tensor_add(out=FM[:], in0=FM[:], in1=t1[:])

                # cumulative count over fused order
                cnt_ps = ps_sm.tile([128, 256], f32, tag="sm")
                nc.tensor.matmul(out=cnt_ps[:], lhsT=triu[:], rhs=FM[:], start=True, stop=False)
                tot_ps = ps_sm.tile([128, 256], f32, tag="sm")
                nc.tensor.matmul(out=tot_ps[:1, :], lhsT=ones128[:],
                                 rhs=FM[:], start=True, stop=True)
                # exclusive prefix over blocks (on partition 0)
                pfA = rpool.tile([1, 256], f32, tag="pfA")
                pfB = rpool.tile([1, 256], f32, tag="pfB")
                tot = rpool.tile([1, 256], f32, tag="tot")
                nc.vector.tensor_copy(out=tot[:], in_=tot_ps[:1, :])
                nc.vector.tensor_copy(out=pfA[:], in_=tot[:])
                cur, nxt = pfA, pfB
                for sh in (1, 2, 4, 8, 16):
                    w = sh * E
                    nc.vector.tensor_add(out=nxt[0:1, w:256], in0=cur[0:1, w:256], in1=cur[0:1, 0:256 - w])
                    nc.vector.tensor_copy(out=nxt[0:1, 0:w], in_=cur[0:1, 0:w])
                    cur, nxt = nxt, cur
                offs = rpool.tile([1, 256], f32, tag="offs")
                nc.vector.tensor_sub(out=offs[:], in0=cur[:], in1=tot[:])
                nc.tensor.matmul(out=cnt_ps[:], lhsT=ones1x[:1, :], rhs=offs[:], start=False, stop=True)

                cnt = rpool.tile([128, BLK * E], f32, tag="cnt")
                nc.vector.tensor_copy(out=cnt[:], in_=cnt_ps[:])
                keep = rpool.tile([128, BLK * E], f32, tag="keep")
                nc.vector.tensor_scalar(out=keep[:], in0=cnt[:], scalar1=CAP + 0.5, scalar2=None, op0=AL.is_le)
                trunc = rpool.tile([128, BLK * E], f32, tag="trunc")
                nc.vector.tensor_mul(out=trunc[:], in0=FM[:], in1=keep[:])
                mske = rpool.tile([128, BLK * E], f32, tag="mske")
                nc.vector.tensor_mul(out=mske[:], in0=trunc[:], in1=eself[:])
                slot1 = rpool.tile([128, BLK * E], f32, tag="slot1")
                nc.vector.tensor_mul(out=slot1[:], in0=cnt[:], in1=mske[:])
                pm = rpool.tile([128, BLK * E], f32, tag="pm")
                nc.vector.tensor_mul(out=pm[:], in0=P2[:].rearrange("p blk e -> p (blk e)"), in1=mske[:])

                def ereduce(dst, srcf, tag):
                    sv = srcf.rearrange("p (blk e) -> p blk e", e=E)
                    nc.vector.tensor_copy(out=dst[:], in_=sv[:, :, 0])
                    for e in range(1, E):
                        nc.vector.tensor_add(out=dst[:], in0=dst[:], in1=sv[:, :, e])

                myslot = rpool.tile([128, BLK], f32, tag="myslot")
                ereduce(myslot, slot1[:], "ms")
                mine = rpool.tile([128, BLK], f32, tag="mine")
                ereduce(mine, mske[:], "mn")
                myprob = rpool.tile([128, BLK], f32, tag="mp")
                ereduce(myprob, pm[:], "mr")

                # payload: [tok-8192, prob]; dest: b*CAP+slot-1 (mine) else DUMP
                nc.vector.tensor_scalar(out=PAY[:, b * BLK:(b + 1) * BLK, 0],
                                        in0=tqio[:], scalar1=float(b * S - T), scalar2=None, op0=AL.add)
                nc.vector.tensor_copy(out=PAY[:, b * BLK:(b + 1) * BLK, 1], in_=myprob[:])
                sd = rpool.tile([128, BLK], f32, tag="sd")
                nc.vector.tensor_scalar(out=sd[:], in0=myslot[:], scalar1=float(b * CAP - 1 - DUMP), scalar2=None, op0=AL.add)
                nc.vector.tensor_mul(out=sd[:], in0=sd[:], in1=mine[:])
                nc.vector.tensor_scalar(out=sd[:], in0=sd[:], scalar1=float(DUMP), scalar2=None, op0=AL.add)
                nc.sync.dma_start(out=scxD[b, :].rearrange("(blk p) -> p blk", p=128), in_=sd[:])

            # inversion scatters (per batch)
            for b in range(B):
                scw = spool.tile([128, NF // 16], f32, tag="scw")
                src = scxD[b, :].rearrange("(j c) -> c j", c=16)
                for r in range(8):
                    nc.sync.dma_start(out=scw[r * 16:(r + 1) * 16, :], in_=src)
                scw16 = spool.tile([128, NF // 16], i16, tag="scw16")
                nc.vector.tensor_copy(out=scw16[:], in_=scw[:])
                nc.gpsimd.dma_scatter_add(
                    out_ap=slotD[:, 0:2], in_ap=PAY[:, b * BLK:(b + 1) * BLK, :],
                    idxs_ap=scw16[:], num_idxs=NF, num_idxs_reg=NF,
                    elem_size=2, elem_step=64, queue_num=1)

            # ---------------- phase 3: FFN per batch-supertile ----------------
            for b in range(B):
                # slot metadata
                gidf = spool.tile([128, CAP // 16], f32, tag="gidf")
                for r in range(8):
                    nc.sync.dma_start(
                        out=gidf[r * 16:(r + 1) * 16, :],
                        in_=slotD[b * CAP:(b + 1) * CAP, 0].rearrange("(j c) -> c j", c=16))
                sidf = spool.tile([128, CAP // 16], f32, tag="sidf")
                nc.vector.tensor_scalar(out=sidf[:], in0=gidf[:], scalar1=float(T), scalar2=None, op0=AL.add)
                sid16 = spool.tile([128, CAP // 16], i16, tag="sid16")
                nc.vector.tensor_copy(out=sid16[:], in_=sidf[:])
                gidc = spool.tile([128, CAP // 16], f32, tag="gidc")
                nc.vector.tensor_scalar(out=gidc[:], in0=sidf[:], scalar1=float(T - 1), scalar2=None, op0=AL.min)
                gid16 = spool.tile([128, CAP // 16], i16, tag="gid16")
                nc.vector.tensor_copy(out=gid16[:], in_=gidc[:])
                prb = spool.tile([128, CAP // 128], f32, tag="prb")
                nc.sync.dma_start(out=prb[:], in_=slotD[b * CAP:(b + 1) * CAP, 1].rearrange("(j p) -> p j", p=128))

                # gather + transpose
                XdT = xdt_pool.tile([128, HC, CAP], f32r, tag="xdt")
                for j in range(CAP // 128):
                    xdn = xd_pool.tile([128, 1, H], f32, tag="xdn")
                    nc.gpsimd.dma_gather(
                        out_ap=xdn[:], in_ap=hsD[:], idxs_ap=gid16[:, j * 8:(j + 1) * 8],
                        num_idxs=128, num_idxs_reg=128, elem_size=H, queue_num=0)
                    for hc in range(HC):
                        tp = ps_tp.tile([128, 128], f32, tag="tp")
                        nc.tensor.transpose(out=tp[:], in_=xdn[:, 0, hc * 128:(hc + 1) * 128], identity=ident[:])
                        nc.vector.tensor_copy(out=XdT[:, hc, j * 128:(j + 1) * 128], in_=tp[:])

                # pass 1: g = silu(x@w1) * (x@w3), fp32r
                g = g_pool.tile([128, FC, CAP], bf16, tag="g")
                for fc in range(FC):
                    w1s = w13s_pool.tile([128, HC, 128], f32, tag="w1s")
                    nc.gpsimd.dma_start(out=w1s[:], in_=w1D[:, fc * 128:(fc + 1) * 128].rearrange("(hc p) f -> p hc f", p=128))
                    w1r = w13r_pool.tile([128, HC, 128], f32r, tag="w1r")
                    nc.scalar.copy(out=w1r[:], in_=w1s[:])
                    w3s = w13s_pool.tile([128, HC, 128], f32, tag="w3s")
                    nc.gpsimd.dma_start(out=w3s[:], in_=w3D[:, fc * 128:(fc + 1) * 128].rearrange("(hc p) f -> p hc f", p=128))
                    w3r = w13r_pool.tile([128, HC, 128], f32r, tag="w3r")
                    nc.scalar.copy(out=w3r[:], in_=w3s[:])
                    for c0 in range(0, CAP, 320):
                        ps1 = ps_mm.tile([128, 320], f32, tag="mm")
                        ps3 = ps_mm.tile([128, 320], f32, tag="mm")
                        for hc in range(HC):
                            nc.tensor.matmul(out=ps1[:], lhsT=w1r[:, hc, :], rhs=XdT[:, hc, c0:c0 + 320],
                                             start=(hc == 0), stop=(hc == HC - 1))
                        for hc in range(HC):
                            nc.tensor.matmul(out=ps3[:], lhsT=w3r[:, hc, :], rhs=XdT[:, hc, c0:c0 + 320],
                                             start=(hc == 0), stop=(hc == HC - 1))
                        sil = xpool.tile([128, 320], f32, tag="sil")
                        nc.scalar.activation(out=sil[:], in_=ps1[:], func=ACTF.Silu)
                        nc.vector.tensor_tensor(out=g[:, fc, c0:c0 + 320], in0=sil[:], in1=ps3[:], op=AL.mult)

                # pass 2: out = g @ w2, transpose back, scale by prob, scatter
                on_t = on_pool.tile([128, CAP // 128, H], f32, tag="on")
                for hc in range(HC):
                    oT = xpool.tile([128, CAP], f32, tag="oT")
                    for half in range(2):
                        fch = FC // 2  # 14
                        w2s = w2s_pool.tile([128, FC // 2, 128], f32, tag="w2s")
                        nc.gpsimd.dma_start(
                            out=w2s[:],
                            in_=w2D[half * fch * 128:(half + 1) * fch * 128, hc * 128:(hc + 1) * 128]
                                .rearrange("(fc p) h -> p fc h", p=128))
                        w2r = w2r_pool.tile([128, FC // 2, 128], bf16, tag="w2r")
                        nc.scalar.copy(out=w2r[:], in_=w2s[:])
                        for c0 in range(0, CAP, 320):
                            pso = ps_mm.tile([128, 320], f32, tag="mm")
                            for fq in range(fch):
                                fc = half * fch + fq
                                nc.tensor.matmul(out=pso[:], lhsT=w2r[:, fq, :], rhs=g[:, fc, c0:c0 + 320],
                                                 start=(fq == 0), stop=(fq == fch - 1))
                            if half == 0:
                                nc.vector.tensor_copy(out=oT[:, c0:c0 + 320], in_=pso[:])
                            else:
                                nc.vector.tensor_add(out=oT[:, c0:c0 + 320], in0=oT[:, c0:c0 + 320], in1=pso[:])
                    for j in range(CAP // 128):
                        tpo = ps_tp.tile([128, 128], f32, tag="tp")
                        otr = xpool.tile([128, 128], f32, tag="otr")
                        nc.vector.tensor_copy(out=otr[:], in_=oT[:, j * 128:(j + 1) * 128])
                        nc.tensor.transpose(out=tpo[:], in_=otr[:], identity=ident[:])
                        nc.vector.tensor_scalar(out=on_t[:, j, hc * 128:(hc + 1) * 128],
                                                in0=tpo[:], scalar1=prb[:, j:j + 1], scalar2=None, op0=AL.mult)
                nc.gpsimd.dma_scatter_add(
                    out_ap=outD[:], in_ap=on_t[:], idxs_ap=sid16[:],
                    num_idxs=CAP, num_idxs_reg=CAP, elem_size=H, queue_num=1)

    nc.finalize()
    return nc


def _host_consts():
    esel = np.zeros((8, 128, BLK * E), np.float32)
    for c in range(8):
        esel[c, :, c::E] = 1.0
    kpar = np.zeros((128, 2), np.float32)
    kpar[0::2, 0] = 1.0
    kpar[1::2, 1] = 1.0
    tq = np.zeros((128, BLK), np.float32)
    p = np.arange(128)
    for blk in range(BLK):
        tq[:, blk] = blk * 64 + p // 2
    return esel, kpar, tq


def kernel(**inputs):
    global _cached
    from concourse.bass_utils import run_bass_kernel_spmd
    if _cached is None:
        _cached = _build()
    nc = _cached

    hs = np.ascontiguousarray(inputs["hidden_states"].reshape(T, H).astype(np.float32, copy=False))
    gw = np.ascontiguousarray(inputs["gate_w"].astype(np.float32, copy=False))
    w1 = inputs["w1"]
    w2 = inputs["w2"]
    w3 = inputs["w3"]
    esel, kpar, tq = _host_consts()

    in_maps = []
    for c in range(8):
        in_maps.append({
            "hs": hs,
            "gate_w": gw,
            "w1e": np.ascontiguousarray(w1[c].astype(np.float32, copy=False)),
            "w2e": np.ascontiguousarray(w2[c].astype(np.float32, copy=False)),
            "w3e": np.ascontiguousarray(w3[c].astype(np.float32, copy=False)),
            "esel": esel[c],
            "kpar": kpar,
            "tqiota": tq,
        })
    import os
    trace = bool(int(os.environ.get("KERNEL_TRACE", "0")))
    kw = {}
    if trace:
        import types
        if "antenv.axon_hooks" not in sys.modules:
            import antenv
            m = types.ModuleType("antenv.axon_hooks")
            holder = {}
            m.set_axon_ntff_profile_hook = lambda h: holder.__setitem__("h", h)
            m.get_axon_ntff_profile_hook = lambda: holder.get("h")
            sys.modules["antenv.axon_hooks"] = m
            antenv.axon_hooks = m
            from trn_agent_boot.trn_boot import _ntff_profile_via_ctypes
            h = _ntff_profile_via_ctypes('/opt/axon/libaxon_pjrt.so')
            m.set_axon_ntff_profile_hook(h)
        kw = dict(trace=True, trace_cores=[int(x) for x in os.environ.get("KERNEL_TRACE_CORES", "0").split(",")], stitch_traces=False)
    res = run_bass_kernel_spmd(nc, in_maps, core_ids=list(range(8)), **kw)
    if trace:
        print("exec_time_ns:", res.exec_time_ns, "mean:", res.mean_exec_time_ns,
              "max core:", res.max_exec_time_core_id)
        if res.per_core_scope_times:
            for scope, d in sorted(res.per_core_scope_times.items()):
                print("scope", scope, {k: f"{v/1000:.0f}us" for k, v in sorted(d.items())})
        if res.instructions_and_trace:
            print("trace:", res.instructions_and_trace[1])
        globals()["last_perf"] = res
    acc = np.zeros((T, H), np.float32)
    for c in range(8):
        acc += res.results[c]["out"][:T]
    return acc.reshape(B, S, H)
